# revision 1
# baseline (speedup 1.0000x reference)
"""GQA causal attention with RoPE, sharded over 8 TRN2 NeuronCores.

Problem: B=1, S=2048, D=2048, H=32 q-heads, KV=8 kv-heads, HD=64.
Sharding: tensor-parallel on kv-heads -- each core owns 1 kv head and its
4 q heads; q/k/v projection weights split column-wise, wo split row-wise.
Each core produces a full (S, D) partial of the output projection; the
host sums the 8 partials (the standard Megatron-TP unshard).

On-chip dataflow is fully transposed ("T-layout", head_dim on partitions)
so that no on-device transposes of activations are needed except tiny
64x128 V tiles:
  qT[j,s]  = MM(lhsT=wq[d,j],  rhs=xT[d,s])     (xT pre-transposed on host)
  kT, vT   likewise from packed wkv
  RoPE applied in T-layout (tables pre-arranged on host)
  scoresT[sk,sq] = MM(lhsT=kT[d,sk], rhs=qT[d,sq])   K=64; even/odd head
                   pairs run concurrently on PE row-groups (0,0)/(64,0)
  pT = exp(scoresT)  (no max subtraction: |scores| <~ 10 so exp is safe)
  outT[d,sq] += MM(lhsT=[v|ones][sk, 65], rhs=pT[sk,sq])  -> row 64 = denom
  attnT = outT[0:64] * recip(denom)  (recip broadcast via gpsimd)
  partial[s,e] += MM(lhsT=attnT[j,s-tile], rhs=wo[j,e])
Causality is exploited at 128-block granularity (upper-left blocks are
never computed); diagonal blocks get an additive triangular mask before
the exp.

PSUM budget (8 banks): "qkv" [128,512] x2 slots (sequential q0/q1/kv
projection chains + v-transpose) = 2 banks, "big" [128,1024] x2 (score
groups + out-projection) = 4 banks, "po" [128,1024] x1 (pv accumulators,
both heads of a pair) = 2 banks.  QKV runs as three sequential 16-MM
chains over a chunk-resident set of 16 x-tiles so each chain needs only
one bank.  Input DMAs alternate between the two HWDGE rings (sync/
scalar); output stores go through SWDGE (gpsimd) to keep rings free.
"""

import os
import numpy as np

import concourse.bass as bass
import concourse.mybir as mybir
import concourse.tile as tile
from concourse import bacc
from concourse.bass_utils import run_bass_kernel_spmd

F32 = mybir.dt.float32

# problem dims (hardcoded per contract)
S, D = 2048, 2048
H, KV, HD = 32, 8, 64
NCORES = 8
QC = (H // NCORES) * HD        # 256 q cols per core (4 heads)
KC = (KV // NCORES) * HD       # 64 kv cols per core (1 kv head)
SCH = 512                      # s-chunk (matmul free dim)
NCH = S // SCH                 # 4 chunks
KT = D // 128                  # 16 contraction tiles
NB = SCH // 128                # 4 sk blocks per chunk
NEG = -1.0e30

# matmul compute dtype: float32r runs the PE at 1 cyc/row (vs 4 for fp32)
# when the moving free dim >= 256.  Overridable for accuracy experiments.
_MM_DT = {"fp32": mybir.dt.float32, "fp32r": mybir.dt.float32r,
          "bf16": mybir.dt.bfloat16}[os.environ.get("KERNEL_MM_DT", "fp32r")]
MMNP = mybir.dt.np(_MM_DT)

_DEBUG = os.environ.get("KERNEL_DEBUG", "") == "1"
_PHASES = os.environ.get("KERNEL_PHASES", "all")  # qkv | attn | all
_ROT_MODE = os.environ.get("KERNEL_ROT_MODE", "dve")  # "dve" | "gpsimd"
_SHIFT_MODE = os.environ.get("KERNEL_SHIFT_MODE", "dve")  # "dve" | "dma"

LAST_RESULTS = None  # BassKernelResults of the most recent run (for test.py)


def _r(ap):
    return ap  # tensors feeding matmuls are declared in _MM_DT directly


def _build_program():
    nc = bacc.Bacc("TRN2", target_bir_lowering=False, debug=False,
                   enable_asserts=False, num_devices=NCORES)

    xT_d = nc.dram_tensor("xT", [D, S], _MM_DT, kind="ExternalInput")
    wq_d = nc.dram_tensor("wq_c", [D, QC], _MM_DT, kind="ExternalInput")
    wkv_d = nc.dram_tensor("wkv_c", [D, 2 * KC], _MM_DT, kind="ExternalInput")
    wo_d = nc.dram_tensor("wo_c", [QC, D], _MM_DT, kind="ExternalInput")
    cos_d = nc.dram_tensor("cosd", [128, S], F32, kind="ExternalInput")
    sin_d = nc.dram_tensor("sind", [128, S], F32, kind="ExternalInput")
    tri_d = nc.dram_tensor("tri", [128, 128], F32, kind="ExternalInput")
    id_d = nc.dram_tensor("ident", [128, KC], _MM_DT, kind="ExternalInput")
    ones_d = nc.dram_tensor("ones", [128, S // 128], _MM_DT, kind="ExternalInput")
    out_d = nc.dram_tensor("part", [S, D], F32, kind="ExternalOutput")

    with tile.TileContext(nc) as tc:
        with (
            tc.tile_pool(name="consts", bufs=1) as consts,
            tc.tile_pool(name="persist", bufs=1) as persist,
            tc.tile_pool(name="xin", bufs=20) as xin,
            tc.tile_pool(name="work", bufs=3) as work,
            tc.tile_pool(name="pt", bufs=6) as ptpool,
            tc.tile_pool(name="outp", bufs=2) as outp,
            tc.tile_pool(name="psbig", bufs=2, space="PSUM") as psbig,
            tc.tile_pool(name="psqkv", bufs=2, space="PSUM") as pskvd,
            tc.tile_pool(name="pso", bufs=2, space="PSUM") as pso,
        ):
            # ---- constants ----
            wq_sb = consts.tile([128, KT, QC], _MM_DT)
            nc.sync.dma_start(wq_sb[:], wq_d.ap().rearrange("(t p) q -> p t q", p=128))
            wkv_sb = consts.tile([128, KT, 2 * KC], _MM_DT)
            nc.sync.dma_start(wkv_sb[:], wkv_d.ap().rearrange("(t p) q -> p t q", p=128))
            wo_sb = consts.tile([128, 2, D], _MM_DT)
            cos_sb = consts.tile([128, S], F32)
            nc.sync.dma_start(cos_sb[:], cos_d.ap())
            sin_sb = consts.tile([128, S], F32)
            nc.sync.dma_start(sin_sb[:], sin_d.ap())
            tri_sb = consts.tile([128, 128], F32)
            nc.sync.dma_start(tri_sb[:], tri_d.ap())
            id_sb = consts.tile([128, KC], _MM_DT)
            nc.sync.dma_start(id_sb[:], id_d.ap())

            # ---- persistent activations (T-layout) ----
            # q stacks: rows 0:64 head 2t, rows 64:128 head 2t+1
            qT = [persist.tile([128, S], _MM_DT, tag=f"qT{t}", name=f"qT{t}") for t in range(2)]
            # kvT: rows 0:64 = kT (after rope), rows 64:128 = vT
            kvT = persist.tile([128, S], _MM_DT, tag="kvT", name="kvT")
            # kodd: rows 64:128 = copy of kT (for row-group-(64,0) matmuls)
            kodd = persist.tile([128, S], _MM_DT, tag="kodd", name="kodd")
            # v in s-major layout with a ones column: per 128-block [128, 65]
            v_sb = persist.tile([128, S // 128, KC + 1], _MM_DT, tag="v", name="v_sb")
            nc.sync.dma_start(v_sb[:, :, KC:KC + 1], ones_d.ap())
            # attention output stacks (divided), same head layout as qT
            aT = [persist.tile([128, S], _MM_DT, tag=f"aT{t}", name=f"aT{t}") for t in range(2)]

            for c in range(NCH):
                cs = bass.ts(c, SCH)  # this chunk's s columns
                _qkv_chunk(nc, c, cs, xT_d, wq_sb, wkv_sb, cos_sb, sin_sb,
                           id_sb, qT, kvT, kodd, v_sb, xin, work, psbig,
                           pskvd)
                if _PHASES in ("attn", "all"):
                    for t in range(2):
                        _attn_pair(nc, c, cs, t, qT[t], kvT, kodd, v_sb,
                                   tri_sb, aT[t], work, ptpool, psbig, pso)
                if _PHASES == "all":
                    if c == 0:
                        nc.scalar.dma_start(
                            wo_sb[:], wo_d.ap().rearrange("(t p) e -> p t e", p=128))
                    _oproj_chunk(nc, c, aT, wo_sb, out_d, outp, psbig)

            if _DEBUG:
                for nm, t_ in [("qT0", qT[0]), ("qT1", qT[1]), ("kvT", kvT),
                               ("kodd", kodd), ("aT0", aT[0]), ("aT1", aT[1])]:
                    dbg = nc.dram_tensor(f"dbg_{nm}", [128, S], _MM_DT,
                                         kind="ExternalOutput")
                    nc.sync.dma_start(dbg.ap(), t_[:])
                dbgv = nc.dram_tensor("dbg_v", [128, S // 128, KC + 1], _MM_DT,
                                      kind="ExternalOutput")
                nc.sync.dma_start(dbgv.ap(), v_sb[:])

    nc.compile()
    return nc


def _qkv_chunk(nc, c, cs, xT_d, wq_sb, wkv_sb, cos_sb, sin_sb, id_sb,
               qT, kvT, kodd, v_sb, xin, work, psbig, pskvd):
    """Project x -> qT/kT/vT for s-chunk c, apply RoPE, build v tiles.

    The 16 x-tiles for the chunk are loaded once (alternating over both
    HWDGE rings) and stay resident; the q0/q1/kv projections then run as
    three sequential 16-MM accumulation chains, each using one PSUM bank.
    """
    xts = []
    for kt in range(KT):
        xt = xin.tile([128, SCH], _MM_DT, tag="xt", name="xt")
        eng = nc.sync if kt % 2 == 0 else nc.scalar
        eng.dma_start(xt[:], xT_d.ap()[bass.ts(kt, 128), cs])
        xts.append(xt)

    def chain(lhs_cols):
        ps = pskvd.tile([128, SCH], F32, tag="qkv", name="ps")
        for kt in range(KT):
            nc.tensor.matmul(ps[:], lhs_cols(kt), xts[kt][:],
                             start=(kt == 0), stop=(kt == KT - 1))
        return ps

    ps0 = chain(lambda kt: wq_sb[:, kt, 0:128])
    _rope(nc, qT[0][:, cs], ps0, cos_sb[:, cs], sin_sb[:, cs], 128, work)
    ps1 = chain(lambda kt: wq_sb[:, kt, 128:256])
    _rope(nc, qT[1][:, cs], ps1, cos_sb[:, cs], sin_sb[:, cs], 128, work)
    ps_kv = chain(lambda kt: wkv_sb[:, kt, :])
    _rope(nc, kvT[0:64, cs], ps_kv[0:64, :], cos_sb[0:64, cs],
          sin_sb[0:64, cs], 64, work)
    # vT rows: plain copy (lanes 64:128 aligned)
    nc.vector.tensor_copy(kvT[64:128, cs], ps_kv[64:128, :])

    # duplicate kT into rows 64:128 of kodd (cross-partition write)
    if _SHIFT_MODE == "dve":
        nc.vector.tensor_copy(kodd[64:128, cs], kvT[0:64, cs])
    else:
        nc.sync.dma_start(kodd[64:128, cs], kvT[0:64, cs])

    # v tiles in s-major layout: transpose vT 64x128 blocks via PE
    for sub in range(NB):
        skb = c * NB + sub
        ps_t = pskvd.tile([128, SCH], _MM_DT, tag="qkv", name="ps")
        nc.tensor.transpose(ps_t[:, 0:KC], kvT[64:128, bass.ts(skb, 128)],
                            id_sb[64:128, :])
        nc.vector.tensor_copy(v_sb[:, skb, 0:KC], ps_t[:, 0:KC])


def _rope(nc, out_sb, ps, cos, sin, rows, work):
    """out = ps * cos + rot_half(ps) * sin   (all in T-layout).

    rot_half swaps 32-row halves within each 64-row head; sin already
    carries the [-s; s] sign pattern.  Cross-partition reads are 32-aligned
    quadrant moves (legal per DVE bank routing).
    """
    tmp = work.tile([128, SCH], F32, tag="ropetmp", name="tmp")
    if _ROT_MODE == "dve":
        for h0 in range(0, rows, 64):
            nc.vector.tensor_mul(tmp[h0:h0 + 32, :], ps[h0 + 32:h0 + 64, :],
                                 sin[h0:h0 + 32, :])
            nc.vector.tensor_mul(tmp[h0 + 32:h0 + 64, :], ps[h0:h0 + 32, :],
                                 sin[h0 + 32:h0 + 64, :])
    else:
        rot = work.tile([128, SCH], F32, tag="roterot", name="rot")
        for h0 in range(0, rows, 64):
            nc.gpsimd.memcpy(rot[h0:h0 + 32, :], ps[h0 + 32:h0 + 64, :])
            nc.gpsimd.memcpy(rot[h0 + 32:h0 + 64, :], ps[h0:h0 + 32, :])
        nc.vector.tensor_mul(tmp[0:rows, :], rot[0:rows, :], sin[0:rows, :])
    nc.vector.tensor_mul(out_sb, ps[0:rows, :], cos[0:rows, :])
    nc.vector.tensor_add(out_sb, out_sb, tmp[0:rows, :])


def _attn_pair(nc, c, cs, t, qTt, kvT, kodd, v_sb, tri_sb, aTt,
               work, ptpool, psbig, pso):
    """Causal attention for head pair (2t, 2t+1) on q chunk c.

    Per 2-block score group and per head: score MMs -> (mask) -> exp ->
    pv-accumulate.  Groups alternate between the even head (PE row-group
    (0,0)) and the odd head ((64,0)), so consecutive MMs use disjoint PE
    rows and overlap.
    """
    nblk = (c + 1) * NB            # causal: sk blocks 0..nblk-1
    d0 = c * NB                    # first diagonal block
    ov = [pso.tile([128, SCH], F32, tag="po", name="ps_o")[0:65, :]
          for _ in range(2)]

    for g in range(0, nblk, 2):
        for hi in range(2):  # head-in-pair
            ps_s = psbig.tile([128, 2 * SCH], F32, tag="big", name="ps")
            for j, b in enumerate((g, g + 1)):
                js = bass.ts(j, SCH)
                if hi == 0:
                    nc.tensor.matmul(ps_s[:, js], _r(kvT[0:64, bass.ts(b, 128)]),
                                     _r(qTt[0:64, cs]), start=True, stop=True)
                else:
                    nc.tensor.matmul(ps_s[:, js], _r(kodd[64:128, bass.ts(b, 128)]),
                                     _r(qTt[64:128, cs]), start=True, stop=True)
            pt = ptpool.tile([128, 2 * SCH], _MM_DT, tag="pt", name="pt")
            if g >= d0:  # diagonal group: per-block mask + exp
                for j, b in enumerate((g, g + 1)):
                    dt_ = b - d0
                    j0 = j * SCH
                    nc.vector.tensor_add(
                        ps_s[:, j0 + dt_ * 128: j0 + (dt_ + 1) * 128],
                        ps_s[:, j0 + dt_ * 128: j0 + (dt_ + 1) * 128],
                        tri_sb[:])
                    nc.scalar.activation(
                        pt[:, j0 + dt_ * 128: j0 + SCH],
                        ps_s[:, j0 + dt_ * 128: j0 + SCH],
                        mybir.ActivationFunctionType.Exp)
            else:        # both blocks full: one wide exp
                nc.scalar.activation(pt[:], ps_s[:],
                                     mybir.ActivationFunctionType.Exp)
            # pv accumulate (+ denominator via the ones column of v)
            for j, b in enumerate((g, g + 1)):
                lo = max(b - d0, 0) * 128
                nc.tensor.matmul(ov[hi][:, lo:SCH],
                                 _r(v_sb[:, b, :]),
                                 _r(pt[:, j * SCH + lo: (j + 1) * SCH]),
                                 start=(b == 0), stop=(b == nblk - 1),
                                 skip_group_check=True)

    # divide by the denominator row and write into the attnT stack
    for hi in range(2):
        recip = work.tile([128, SCH], F32, tag="recip", bufs=2, name="recip")
        # lane-shift the denominator row to partition 0: HW partition_broadcast
        # always reads physical partition 0, ignoring the AP offset (probe3)
        nc.vector.reciprocal(recip[0:1, :], ov[hi][64:65, :])
        bc = work.tile([128, SCH], F32, tag="bcast", bufs=2, name="bc")
        nc.gpsimd.partition_broadcast(bc[0:64, :], recip[0:1, :])
        dst = aTt[0:64, cs] if hi == 0 else aTt[64:128, cs]
        if hi == 0 or _SHIFT_MODE == "dve":
            nc.vector.tensor_mul(dst, ov[hi][0:64, :], bc[0:64, :])
        else:
            stg = work.tile([128, SCH], F32, tag="stg", bufs=2, name="stg")
            nc.vector.tensor_mul(stg[0:64, :], ov[hi][0:64, :], bc[0:64, :])
            nc.sync.dma_start(dst, stg[0:64, :])


def _oproj_chunk(nc, c, aT, wo_sb, out_d, outp, psbig):
    """partial[s,e] = sum_j attnT[j,s] * wo[j,e] for this chunk's s rows."""
    for st in range(NB):
        srow = c * NB + st
        osb = outp.tile([128, D], F32, tag="osb", name="osb")
        for eh in range(D // (2 * SCH)):
            ps = psbig.tile([128, 2 * SCH], F32, tag="big", name="ps")
            for j in range(2):
                ec = 2 * eh + j
                nc.tensor.matmul(ps[:, bass.ts(j, SCH)],
                                 _r(aT[0][:, bass.ts(srow, 128)]),
                                 _r(wo_sb[:, 0, bass.ts(ec, SCH)]),
                                 start=True, stop=False)
                nc.tensor.matmul(ps[:, bass.ts(j, SCH)],
                                 _r(aT[1][:, bass.ts(srow, 128)]),
                                 _r(wo_sb[:, 1, bass.ts(ec, SCH)]),
                                 start=False, stop=True)
            if eh % 2 == 0:
                nc.vector.tensor_copy(osb[:, bass.ts(eh, 2 * SCH)], ps[:])
            else:
                nc.scalar.copy(osb[:, bass.ts(eh, 2 * SCH)], ps[:])
        nc.gpsimd.dma_start(out_d.ap()[bass.ts(srow, 128), :], osb[:])


_program_cache = None


def _get_program():
    global _program_cache
    if _program_cache is None:
        _program_cache = _build_program()
    return _program_cache


def bench_ns(ins, iters=20, warmup=3):
    """Time the SPMD kernel with device-resident inputs; returns ns/iter.

    Replicates bass2jax.run_bass_via_pjrt's shard_map jit but keeps the
    jitted executable and the device inputs alive across calls, so the
    per-iteration wall clock approximates device execution time (the axon
    NTFF profiling hook is unavailable in this container).
    """
    import time
    import jax
    from jax.sharding import Mesh, PartitionSpec, NamedSharding
    from jax.experimental.shard_map import shard_map
    from concourse import bass2jax
    import concourse.mybir as mybir_

    bass2jax.install_neuronx_cc_hook()
    nc = _get_program()

    pid_name = nc.partition_id_tensor.name if nc.partition_id_tensor else None
    in_names, out_names, out_avals = [], [], []
    for alloc in nc.m.functions[0].allocations:
        if not isinstance(alloc, mybir_.MemoryLocationSet):
            continue
        name = alloc.memorylocations[0].name
        if alloc.kind == "ExternalInput":
            if name != pid_name:
                in_names.append(name)
        elif alloc.kind == "ExternalOutput":
            out_names.append(name)
            out_avals.append(jax.core.ShapedArray(
                tuple(alloc.tensor_shape), mybir_.dt.np(alloc.dtype)))
    n_params = len(in_names)
    all_names = in_names + out_names
    if pid_name is not None:
        all_names = all_names + [pid_name]

    def _body(*args):
        operands = list(args)
        if pid_name is not None:
            operands.append(bass2jax.partition_id_tensor())
        outs = bass2jax._bass_exec_p.bind(
            *operands, out_avals=tuple(out_avals), in_names=tuple(all_names),
            out_names=tuple(out_names), lowering_input_output_aliases=(),
            sim_require_finite=True, sim_require_nnan=True, nc=nc)
        return tuple(outs)

    devices = jax.devices()[:NCORES]
    mesh = Mesh(np.asarray(devices), ("core",))
    nin = n_params + len(out_names)
    sharded = jax.jit(
        shard_map(_body, mesh=mesh, in_specs=(PartitionSpec("core"),) * nin,
                  out_specs=(PartitionSpec("core"),) * len(out_names),
                  check_rep=False),
        keep_unused=True)

    sh = NamedSharding(mesh, PartitionSpec("core"))
    dev_args = [
        jax.device_put(
            np.concatenate([np.asarray(ins[c][nm]) for c in range(NCORES)], 0), sh)
        for nm in in_names
    ] + [
        jax.device_put(np.zeros((NCORES * av.shape[0], *av.shape[1:]), av.dtype), sh)
        for av in out_avals
    ]

    for _ in range(warmup):
        out = sharded(*dev_args)
    jax.block_until_ready(out)
    t0 = time.perf_counter()
    for _ in range(iters):
        out = sharded(*dev_args)
    jax.block_until_ready(out)
    return (time.perf_counter() - t0) / iters * 1e9


def kernel(x, rope_cos, rope_sin, wq, wk, wv, wo):
    global LAST_RESULTS
    args = [np.asarray(a, dtype=np.float32)
            for a in (x, rope_cos, rope_sin, wq, wk, wv, wo)]
    ins = build_inputs(*args)
    nc = _get_program()
    LAST_RESULTS = run_bass_kernel_spmd(nc, ins, core_ids=list(range(NCORES)))
    parts = [r["part"] for r in LAST_RESULTS.results]
    out = parts[0].astype(np.float64)
    for p in parts[1:]:
        out += p
    return out.astype(np.float32)[None]


def build_inputs(x, rope_cos, rope_sin, wq, wk, wv, wo):
    """Shard + lay out the full inputs into the 8 per-core input maps."""
    xT = np.ascontiguousarray(x.reshape(S, D).T)            # (D, S)
    cos64 = np.concatenate([rope_cos.T, rope_cos.T], 0)     # (64, S)
    sin64 = np.concatenate([-rope_sin.T, rope_sin.T], 0)    # (64, S)
    cosd = np.ascontiguousarray(np.tile(cos64, (2, 1)))     # (128, S)
    sind = np.ascontiguousarray(np.tile(sin64, (2, 1)))
    sk = np.arange(128)[:, None]
    sq = np.arange(128)[None, :]
    tri = np.where(sk <= sq, 0.0, NEG).astype(np.float32)   # (128,128)
    ident = np.tile(np.eye(KC, dtype=np.float32), (2, 1)).astype(MMNP)   # (128, 64)

    ins = []
    for cidx in range(NCORES):
        qs = slice(cidx * QC, (cidx + 1) * QC)
        ks = slice(cidx * KC, (cidx + 1) * KC)
        ins.append({
            "xT": xT.astype(MMNP),
            # fold the attention scale into wq (RoPE is linear, so it commutes)
            "wq_c": (np.ascontiguousarray(wq[:, qs]) * np.float32(HD ** -0.5)).astype(MMNP),
            "wkv_c": np.ascontiguousarray(
                np.concatenate([wk[:, ks], wv[:, ks]], axis=1)).astype(MMNP),
            "wo_c": np.ascontiguousarray(wo[qs, :]).astype(MMNP),
            "cosd": cosd,
            "sind": sind,
            "tri": tri,
            "ident": ident,
            "ones": np.ones((128, S // 128), dtype=MMNP),
        })
    return ins

